# revision 1
# baseline (speedup 1.0000x reference)
"""Trainium2 Bass kernel: ADI implicit diffusion layer (nn_DiffusionLayer).

Math: per time step the reference does three tridiagonal (Thomas) solves
(x-dir dt/2, y-dir dt, x-dir dt/2) followed by a 3x3 channel coupling, and a
final sigmoid-skip blend.  All tridiagonal coefficient fields depend only on
the (C,H,W) parameter tensors and the (compile-time) step index, never on u.

Key transformations used here:
  * Thomas forward/backward sweeps are first-order linear recurrences once the
    pivot denominators are known -> one hardware `tensor_tensor_scan` each
    (the backward sweep uses reversed access patterns).
  * The pivot denominators obey denom_i = B_i - g_i * g_{i-1} / denom_{i-1}.
    Because g ~ alpha*dt/dx^2 <= ~5e-3, truncating the continued fraction at
    depth 1 (denom_i ~= B_i - g_i*g_{i-1}) is exact to < 1e-8 relative, and
    1/denom is computed with the polynomial 1 - d + d^2 (d = denom-1), also
    exact at fp32 for these magnitudes.  This removes every sequential
    dependency from the coefficient preparation (validated bit-exact vs the
    jax reference on the provided inputs, ~5e-7 rel with randn coupling).
  * Data parallel over batch: 32 batches -> 4 per NeuronCore, zero collectives.

Layout per core: state tiles (128, 4, 1536) fp32 where partition = h (mod 128)
and free = (b, c*2+h_half, w).  The y-direction solves run on a PE-transposed
copy with the roles of h and w exchanged.
"""

import sys

if "/opt/trn_rl_repo" not in sys.path:
    sys.path.insert(0, "/opt/trn_rl_repo")

from contextlib import ExitStack

import numpy as np

import concourse.bass as bass
import concourse.bacc as bacc
from concourse import mybir
from concourse.bass_utils import run_bass_kernel_spmd
from concourse.tile import TileContext
from concourse.masks import make_identity

NCORES = 8
B, C, N = 32, 3, 256
BL = B // NCORES          # batches per core
S = 2 * C                 # (c, h-half) slices stacked along the free axis
F = S * N                 # free elements per batch row group
DT, NUM_STEPS, EPS = 0.01, 10, 1e-6
f32 = mybir.dt.float32
OP = mybir.AluOpType

_cached_nc = {}


def _chw_ap(dram_h, b=None):
    """(128, C, 2, N) access pattern over a (C,N,N) or (BL,C,N,N) DRAM tensor
    with partition = h mod 128."""
    off = 0 if b is None else b * C * N * N
    return bass.AP(tensor=dram_h, offset=off,
                   ap=[[N, 128], [N * N, C], [128 * N, 2], [1, N]])


def _build(fast=True, diag=False):
    """diag=True: channel_coupling is diagonal, so it commutes with the
    per-channel tridiagonal solves; all couplings collapse into per-channel
    scalars applied with the final blend (host puts (1-s)*m_c^NUM_STEPS into
    scal columns 9..11)."""
    nc = bacc.Bacc("TRN2", target_bir_lowering=False, debug=False,
                   num_devices=NCORES)
    u_d = nc.dram_tensor("u", [BL, C, N, N], f32, kind="ExternalInput")
    ab_d = nc.dram_tensor("ab", [C, N, N], f32, kind="ExternalInput")
    atc_d = nc.dram_tensor("atc", [C, N, N], f32, kind="ExternalInput")
    bbt_d = nc.dram_tensor("bbt", [C, N, N], f32, kind="ExternalInput")
    btct_d = nc.dram_tensor("btct", [C, N, N], f32, kind="ExternalInput")
    scal_d = nc.dram_tensor("scal", [128, 24], f32, kind="ExternalInput")
    y_d = nc.dram_tensor("y", [BL, C, N, N], f32, kind="ExternalOutput")

    with TileContext(nc) as tc, ExitStack() as ctx:
        consts = ctx.enter_context(tc.tile_pool(name="consts", bufs=1))
        statep = ctx.enter_context(tc.tile_pool(name="state", bufs=1))
        fieldsp = ctx.enter_context(tc.tile_pool(name="fields", bufs=1))
        scr = ctx.enter_context(tc.tile_pool(name="scr", bufs=1))
        psum = ctx.enter_context(tc.tile_pool(name="psum", bufs=4, space="PSUM"))

        V = nc.vector

        ident = consts.tile([128, 128], f32, tag="ident")
        make_identity(nc, ident)
        scal = consts.tile([128, 24], f32, tag="scal")
        nc.sync.dma_start(out=scal[:, :], in_=scal_d.ap())

        ab = consts.tile([128, F], f32, tag="ab")
        atc = consts.tile([128, F], f32, tag="atc")
        bbt = consts.tile([128, F], f32, tag="bbt")
        btct = consts.tile([128, F], f32, tag="btct")
        for t_, d_ in ((ab, ab_d), (atc, atc_d), (bbt, bbt_d), (btct, btct_d)):
            nc.sync.dma_start(out=t_[:, :], in_=_chw_ap(d_))

        A = statep.tile([128, BL, F], f32, tag="A")
        Bt = statep.tile([128, BL, F], f32, tag="B")
        for b in range(BL):
            nc.sync.dma_start(out=A[:, b], in_=_chw_ap(u_d, b))

        def fset(tag):
            return dict(
                r=fieldsp.tile([128, F], f32, tag=tag + "r", name=tag + "r"),
                pf=fieldsp.tile([128, F], f32, tag=tag + "pf", name=tag + "pf"),
                pb=fieldsp.tile([128, F], f32, tag=tag + "pb", name=tag + "pb"))

        fx = [fset("fx0"), fset("fx1")]
        fy = fset("fy")
        ct = scr.tile([128, F], f32, tag="ct")
        g = scr.tile([128, F], f32, tag="g")
        tmp = scr.tile([128, F], f32, tag="tmp")
        dl = scr.tile([128, F], f32, tag="dl")
        s_t = scr.tile([128, F], f32, tag="s_t")
        e_t = scr.tile([128, F], f32, tag="e_t")

        AF = mybir.ActivationFunctionType

        def smooth_into(dst, src, dtf):
            """dst = moving-average(src, replicate pad per row) * dtf."""
            V.tensor_tensor(tmp[:, 1:F - 1], src[:, 0:F - 2], src[:, 2:F], OP.add)
            V.tensor_tensor(dst[:, 1:F - 1], tmp[:, 1:F - 1], src[:, 1:F - 1], OP.add)
            V.scalar_tensor_tensor(dst[:, 0::N], src[:, 0::N], 2.0, src[:, 1::N],
                                   OP.mult, OP.add)
            V.scalar_tensor_tensor(dst[:, N - 1::N], src[:, N - 1::N], 2.0,
                                   src[:, N - 2::N], OP.mult, OP.add)
            nc.scalar.mul(dst[:, :], dst[:, :], float(dtf))

        if fast:
            eps_b = consts.tile([128, 1], f32, tag="eps_b")
            one_b = consts.tile([128, 1], f32, tag="one_b")
            mhalf_b = consts.tile([128, 1], f32, tag="mhalf_b")
            b34 = consts.tile([128, 1], f32, tag="b34")
            V.memset(eps_b[:, :], float(EPS))
            V.memset(one_b[:, :], 1.0)
            V.memset(mhalf_b[:, :], -0.5)
            V.memset(b34[:, :], 0.75)
            # coefficient fields: alpha_t never clamps (host-verified), so
            # g(t) = g0 + t*g1 with one-time smoothed coefficient tiles.
            g0x = fieldsp.tile([128, F], f32, tag="g0x")
            g1x = fieldsp.tile([128, F], f32, tag="g1x")
            g0y = fieldsp.tile([128, F], f32, tag="g0y")
            g1y = fieldsp.tile([128, F], f32, tag="g1y")
            smooth_into(g0x, ab, DT / 6.0)
            smooth_into(g1x, atc, DT / 6.0)
            smooth_into(g0y, bbt, DT / 3.0)
            smooth_into(g1y, btct, DT / 3.0)

        def prep_fast(fs, g0, g1, t):
            # g = g0 + t*g1  (DVE)
            V.scalar_tensor_tensor(g[:, :], g1[:, :], float(t), g0[:, :],
                                   OP.mult, OP.add)
            # dl = denom-1 = 2g+EPS interior, g+EPS at row boundaries  (ACT)
            nc.scalar.activation(dl[:, :], g[:, :], AF.Identity,
                                 bias=eps_b[:, 0:1], scale=2.0)
            nc.scalar.activation(dl[:, 0::N], g[:, 0::N], AF.Identity,
                                 bias=eps_b[:, 0:1], scale=1.0)
            nc.scalar.activation(dl[:, N - 1::N], g[:, N - 1::N], AF.Identity,
                                 bias=eps_b[:, 0:1], scale=1.0)
            # r = 1 - dl + dl^2 == (dl - 0.5)^2 + 0.75, entirely on ScalarE
            nc.scalar.activation(tmp[:, :], dl[:, :], AF.Square,
                                 bias=mhalf_b[:, 0:1], scale=1.0)
            nc.scalar.activation(fs["r"][:, :], tmp[:, :], AF.Identity,
                                 bias=b34[:, 0:1], scale=1.0)
            V.tensor_mul(fs["pb"][:, :], g[:, :], fs["r"][:, :])
            nc.scalar.copy(fs["pf"][:, :], fs["pb"][:, :])
            V.memset(fs["pb"][:, N - 1::N], 0.0)
            V.memset(fs["pf"][:, 0::N], 0.0)

        def prep(fs, base, tcoef, t, dt_eff):
            """Build r = 1/denom and the scan coefficient fields phi = g*r."""
            dtf = dt_eff / 3.0
            # ct = max((tcoef*t + base) * dtf, EPS*dtf); the /3 of the moving
            # average and the dt/dx^2 scale are folded in up front.
            V.scalar_tensor_tensor(ct[:, :], tcoef[:, :], float(t), base[:, :],
                                   OP.mult, OP.add)
            V.tensor_scalar(ct[:, :], ct[:, :], float(dtf), float(EPS * dtf),
                            OP.mult, OP.max)
            # g = moving-average smooth along the solve axis (replicate pad).
            V.tensor_tensor(tmp[:, 1:F - 1], ct[:, 0:F - 2], ct[:, 2:F], OP.add)
            V.tensor_tensor(g[:, 1:F - 1], tmp[:, 1:F - 1], ct[:, 1:F - 1], OP.add)
            V.scalar_tensor_tensor(g[:, 0::N], ct[:, 0::N], 2.0, ct[:, 1::N],
                                   OP.mult, OP.add)
            V.scalar_tensor_tensor(g[:, N - 1::N], ct[:, N - 1::N], 2.0,
                                   ct[:, N - 2::N], OP.mult, OP.add)
            # dl = denom - 1 = 2g + EPS - g_i*g_{i-1}, with boundary rows of the
            # tridiagonal having diag 1+g instead of 1+2g.
            V.tensor_scalar(dl[:, :], g[:, :], 2.0, float(EPS), OP.mult, OP.add)
            V.tensor_tensor(tmp[:, 1:F], g[:, 1:F], g[:, 0:F - 1], OP.mult)
            V.tensor_sub(dl[:, 1:F], dl[:, 1:F], tmp[:, 1:F])
            V.tensor_scalar(dl[:, 0::N], g[:, 0::N], float(EPS), None, OP.add)
            V.tensor_sub(dl[:, N - 1::N], dl[:, N - 1::N], g[:, N - 1::N])
            # r = 1 - dl*(1 - dl)
            V.tensor_scalar(tmp[:, :], dl[:, :], -1.0, 1.0, OP.mult, OP.add)
            V.tensor_mul(tmp[:, :], dl[:, :], tmp[:, :])
            V.tensor_scalar(fs["r"][:, :], tmp[:, :], -1.0, 1.0, OP.mult, OP.add)
            # phi = g*r; forward variant zeroes row starts, backward row ends.
            V.tensor_mul(fs["pb"][:, :], g[:, :], fs["r"][:, :])
            nc.scalar.copy(fs["pf"][:, :], fs["pb"][:, :])
            V.memset(fs["pb"][:, N - 1::N], 0.0)
            V.memset(fs["pf"][:, 0::N], 0.0)

        def solve(buf, fs):
            for b in range(BL):
                bb = buf[:, b]
                V.tensor_mul(s_t[:, :], bb, fs["r"][:, :])
                V.tensor_tensor_scan(e_t[:, :], fs["pf"][:, :], s_t[:, :], 0.0,
                                     OP.mult, OP.add)
                V.tensor_tensor_scan(bb[:, ::-1], fs["pb"][:, ::-1],
                                     e_t[:, ::-1], 0.0, OP.mult, OP.add)

        def transpose_vol(src, dst):
            for b in range(BL):
                for c in range(C):
                    pt = psum.tile([128, 512], f32, tag="pt")
                    for sb in range(2):
                        for db in range(2):
                            nc.tensor.transpose(
                                pt[:, db * 256 + sb * 128:db * 256 + (sb + 1) * 128],
                                src[:, b, (c * 2 + sb) * N + db * 128:
                                    (c * 2 + sb) * N + (db + 1) * 128],
                                ident[:, :])
                    nc.scalar.copy(dst[:, b, c * 2 * N:(c * 2 + 2) * N], pt[:, :])

        def couple(src, dst, mbase):
            srcv = src[:, :, :].rearrange("p b (c x) -> p b c x", c=C)
            dstv = dst[:, :, :].rearrange("p b (c x) -> p b c x", c=C)
            for d in range(C):
                nc.scalar.mul(dstv[:, :, d, :], srcv[:, :, 0, :],
                              scal[:, mbase + d * 3:mbase + d * 3 + 1])
                for cc in range(1, C):
                    V.scalar_tensor_tensor(
                        dstv[:, :, d, :], srcv[:, :, cc, :],
                        scal[:, mbase + d * 3 + cc:mbase + d * 3 + cc + 1],
                        dstv[:, :, d, :], OP.mult, OP.add)

        def prep_x(fs, t):
            if fast:
                prep_fast(fs, g0x, g1x, t)
            else:
                prep(fs, ab, atc, t, DT / 2)

        def prep_y(fs, t):
            if fast:
                prep_fast(fs, g0y, g1y, t)
            else:
                prep(fs, bbt, btct, t, DT)

        cur, other = A, Bt
        xcache_t = None
        xping = 0
        t = 0.0
        for step in range(NUM_STEPS):
            if xcache_t != t:
                xping ^= 1
                prep_x(fx[xping], t)
                xcache_t = t
            solve(cur, fx[xping])
            t += DT / 2
            prep_y(fy, t)
            transpose_vol(cur, other)
            solve(other, fy)
            transpose_vol(other, cur)
            t += DT / 2
            if xcache_t != t:
                xping ^= 1
                prep_x(fx[xping], t)
                xcache_t = t
            solve(cur, fx[xping])
            if not diag:
                mbase = 0 if step < NUM_STEPS - 1 else 9
                couple(cur, other, mbase)
                cur, other = other, cur

        # skip blend: out = s*orig + (1-s)*u_final.  Generic path folded (1-s)
        # into the last coupling matrix; diag path applies the commuted
        # per-channel factor (1-s)*m_c^NUM_STEPS here instead.
        for b in range(BL):
            nc.sync.dma_start(out=other[:, b], in_=_chw_ap(u_d, b))
        if diag:
            for b in range(BL):
                for c in range(C):
                    sl = slice(c * 2 * N, (c + 1) * 2 * N)
                    nc.scalar.mul(cur[:, b, sl], cur[:, b, sl],
                                  scal[:, 20 + c:21 + c])
        for b in range(BL):
            V.scalar_tensor_tensor(other[:, b], other[:, b], scal[:, 18:19],
                                   cur[:, b], OP.mult, OP.add)
            nc.sync.dma_start(out=_chw_ap(y_d, b), in_=other[:, b])

    nc.compile()
    return nc


def _fast_ok(alpha_base, beta_base, alpha_time_coeff, beta_time_coeff):
    """Fast path assumes max(base + t*coef, EPS) never clamps for any solve
    time t in [0, NUM_STEPS*DT]."""
    tmax = NUM_STEPS * DT
    for base, tc in ((alpha_base, alpha_time_coeff), (beta_base, beta_time_coeff)):
        base = np.asarray(base, np.float64)
        tc = np.asarray(tc, np.float64)
        lo = np.minimum(base, np.minimum(base + tmax * tc, base + 0.005 * tc))
        if lo.min() <= 10 * EPS:
            return False
    return True


def _run(in_maps, trace=False, fast=True, diag=False, **kw):
    key = (fast, diag)
    if key not in _cached_nc:
        _cached_nc[key] = _build(fast, diag)
    return run_bass_kernel_spmd(_cached_nc[key], in_maps, list(range(NCORES)),
                                trace=trace, **kw)


def _make_in_maps(u, alpha_base, beta_base, alpha_time_coeff, beta_time_coeff,
                  channel_coupling, skip_weight):
    u = np.ascontiguousarray(np.asarray(u, np.float32))
    s = 1.0 / (1.0 + np.exp(-float(np.asarray(skip_weight, np.float64))))
    M = np.asarray(channel_coupling, np.float32)
    row = np.zeros(24, np.float32)
    row[0:9] = M.reshape(-1)
    row[9:18] = (np.float32(1.0 - s) * M).reshape(-1)
    row[18] = np.float32(s)
    row[20:23] = np.float32(1.0 - s) * (np.diag(M).astype(np.float64)
                                        ** NUM_STEPS).astype(np.float32)
    scal = np.ascontiguousarray(np.tile(row[None, :], (128, 1)))
    ab = np.ascontiguousarray(np.asarray(alpha_base, np.float32))
    atc = np.ascontiguousarray(np.asarray(alpha_time_coeff, np.float32))
    bbt = np.ascontiguousarray(np.swapaxes(np.asarray(beta_base, np.float32), 1, 2))
    btct = np.ascontiguousarray(np.swapaxes(np.asarray(beta_time_coeff, np.float32), 1, 2))
    return [dict(u=np.ascontiguousarray(u[i * BL:(i + 1) * BL]), ab=ab, atc=atc,
                 bbt=bbt, btct=btct, scal=scal) for i in range(NCORES)]


def _diag_ok(channel_coupling):
    M = np.asarray(channel_coupling, np.float64)
    return bool(np.all(M == np.diag(np.diag(M))))


def kernel(u, alpha_base, beta_base, alpha_time_coeff, beta_time_coeff,
           channel_coupling, skip_weight):
    in_maps = _make_in_maps(u, alpha_base, beta_base, alpha_time_coeff,
                            beta_time_coeff, channel_coupling, skip_weight)
    fast = _fast_ok(alpha_base, beta_base, alpha_time_coeff, beta_time_coeff)
    diag = _diag_ok(channel_coupling)
    res = _run(in_maps, fast=fast, diag=diag)
    return np.concatenate([res.results[i]["y"] for i in range(NCORES)], axis=0)



# revision 6
# speedup vs baseline: 24.9380x; 24.9380x over previous
"""Trainium2 Bass kernel: ADI implicit diffusion layer (nn_DiffusionLayer).

Math: per time step the reference does three tridiagonal (Thomas) solves
(x-dir dt/2, y-dir dt, x-dir dt/2) followed by a 3x3 channel coupling, and a
final sigmoid-skip blend.  All tridiagonal coefficient fields depend only on
the (C,H,W) parameter tensors and the (compile-time) step index, never on u.

Key transformations used here:
  * Thomas forward/backward sweeps are first-order linear recurrences once the
    pivot denominators are known -> one hardware `tensor_tensor_scan` each
    (the backward sweep uses reversed access patterns).
  * The pivot denominators obey denom_i = B_i - g_i * g_{i-1} / denom_{i-1}.
    Because g ~ alpha*dt/dx^2 <= ~5e-3, truncating the continued fraction at
    depth 1 (denom_i ~= B_i - g_i*g_{i-1}) is exact to < 1e-8 relative, and
    1/denom is computed with the polynomial 1 - d + d^2 (d = denom-1), also
    exact at fp32 for these magnitudes.  This removes every sequential
    dependency from the coefficient preparation (validated bit-exact vs the
    jax reference on the provided inputs, ~5e-7 rel with randn coupling).
  * Data parallel over batch: 32 batches -> 4 per NeuronCore, zero collectives.

Layout per core: state tiles (128, 4, 1536) fp32 where partition = h (mod 128)
and free = (b, c*2+h_half, w).  The y-direction solves run on a PE-transposed
copy with the roles of h and w exchanged.
"""

import sys

if "/opt/trn_rl_repo" not in sys.path:
    sys.path.insert(0, "/opt/trn_rl_repo")

from contextlib import ExitStack

import numpy as np
import ml_dtypes

import concourse.bass as bass
import concourse.bacc as bacc
from concourse import mybir
from concourse.bass_utils import run_bass_kernel_spmd
from concourse.tile import TileContext
from concourse.masks import make_identity

NCORES = 8
B, C, N = 32, 3, 256
BL = B // NCORES          # batches per core
S = 2 * C                 # (c, h-half) slices stacked along the free axis
F = S * N                 # free elements per batch row group
DT, NUM_STEPS, EPS = 0.01, 10, 1e-6
DX = 1.0
f32 = mybir.dt.float32
bf16 = mybir.dt.bfloat16
OP = mybir.AluOpType

_cached_nc = {}


def _chw_ap(dram_h, b=None):
    """(128, C, 2, N) access pattern over a (C,N,N) or (BL,C,N,N) DRAM tensor
    with partition = h mod 128."""
    off = 0 if b is None else b * C * N * N
    return bass.AP(tensor=dram_h, offset=off,
                   ap=[[N, 128], [N * N, C], [128 * N, 2], [1, N]])


def _build(fast=True, diag=False):
    """diag=True: channel_coupling is diagonal, so it commutes with the
    per-channel tridiagonal solves; all couplings collapse into per-channel
    scalars applied with the final blend (host puts (1-s)*m_c^NUM_STEPS into
    scal columns 9..11)."""
    nc = bacc.Bacc("TRN2", target_bir_lowering=False, debug=False,
                   num_devices=NCORES)
    u_d = nc.dram_tensor("u", [BL, C, N, N], f32, kind="ExternalInput")
    ab_d = nc.dram_tensor("ab", [C, N, N], f32, kind="ExternalInput")
    atc_d = nc.dram_tensor("atc", [C, N, N], f32, kind="ExternalInput")
    bbt_d = nc.dram_tensor("bbt", [C, N, N], f32, kind="ExternalInput")
    btct_d = nc.dram_tensor("btct", [C, N, N], f32, kind="ExternalInput")
    scal_d = nc.dram_tensor("scal", [128, 24], f32, kind="ExternalInput")
    y_d = nc.dram_tensor("y", [BL, C, N, N], f32, kind="ExternalOutput")

    with TileContext(nc) as tc, ExitStack() as ctx:
        consts = ctx.enter_context(tc.tile_pool(name="consts", bufs=1))
        statep = ctx.enter_context(tc.tile_pool(name="state", bufs=1))
        fieldsp = ctx.enter_context(tc.tile_pool(name="fields", bufs=1))
        scr = ctx.enter_context(tc.tile_pool(name="scr", bufs=1))
        psum = ctx.enter_context(tc.tile_pool(name="psum", bufs=4, space="PSUM"))

        V = nc.vector

        ident = consts.tile([128, 128], f32, tag="ident")
        make_identity(nc, ident)
        scal = consts.tile([128, 24], f32, tag="scal")
        nc.sync.dma_start(out=scal[:, :], in_=scal_d.ap())

        ab = consts.tile([128, F], f32, tag="ab")
        atc = consts.tile([128, F], f32, tag="atc")
        bbt = consts.tile([128, F], f32, tag="bbt")
        btct = consts.tile([128, F], f32, tag="btct")
        for t_, d_ in ((ab, ab_d), (atc, atc_d), (bbt, bbt_d), (btct, btct_d)):
            nc.sync.dma_start(out=t_[:, :], in_=_chw_ap(d_))

        A = statep.tile([128, BL, F], f32, tag="A")
        Bt = statep.tile([128, BL, F], f32, tag="B")
        for b in range(BL):
            nc.sync.dma_start(out=A[:, b], in_=_chw_ap(u_d, b))

        def fset(tag):
            return dict(
                r=fieldsp.tile([128, F], f32, tag=tag + "r", name=tag + "r"),
                pf=fieldsp.tile([128, F], f32, tag=tag + "pf", name=tag + "pf"),
                pb=fieldsp.tile([128, F], f32, tag=tag + "pb", name=tag + "pb"))

        fx = [fset("fx0"), fset("fx1")]
        fy = fset("fy")
        ct = scr.tile([128, F], f32, tag="ct")
        g = scr.tile([128, F], f32, tag="g")
        tmp = scr.tile([128, F], f32, tag="tmp")
        dl = scr.tile([128, F], f32, tag="dl")
        s_t = scr.tile([128, F], f32, tag="s_t")
        e_t = scr.tile([128, F], f32, tag="e_t")

        AF = mybir.ActivationFunctionType

        def smooth_into(dst, src, dtf):
            """dst = moving-average(src, replicate pad per row) * dtf."""
            V.tensor_tensor(tmp[:, 1:F - 1], src[:, 0:F - 2], src[:, 2:F], OP.add)
            V.tensor_tensor(dst[:, 1:F - 1], tmp[:, 1:F - 1], src[:, 1:F - 1], OP.add)
            V.scalar_tensor_tensor(dst[:, 0::N], src[:, 0::N], 2.0, src[:, 1::N],
                                   OP.mult, OP.add)
            V.scalar_tensor_tensor(dst[:, N - 1::N], src[:, N - 1::N], 2.0,
                                   src[:, N - 2::N], OP.mult, OP.add)
            nc.scalar.mul(dst[:, :], dst[:, :], float(dtf))

        if fast:
            eps_b = consts.tile([128, 1], f32, tag="eps_b")
            one_b = consts.tile([128, 1], f32, tag="one_b")
            mhalf_b = consts.tile([128, 1], f32, tag="mhalf_b")
            b34 = consts.tile([128, 1], f32, tag="b34")
            V.memset(eps_b[:, :], float(EPS))
            V.memset(one_b[:, :], 1.0)
            V.memset(mhalf_b[:, :], -0.5)
            V.memset(b34[:, :], 0.75)
            # coefficient fields: alpha_t never clamps (host-verified), so
            # g(t) = g0 + t*g1 with one-time smoothed coefficient tiles.
            g0x = fieldsp.tile([128, F], f32, tag="g0x")
            g1x = fieldsp.tile([128, F], f32, tag="g1x")
            g0y = fieldsp.tile([128, F], f32, tag="g0y")
            g1y = fieldsp.tile([128, F], f32, tag="g1y")
            smooth_into(g0x, ab, DT / 6.0)
            smooth_into(g1x, atc, DT / 6.0)
            smooth_into(g0y, bbt, DT / 3.0)
            smooth_into(g1y, btct, DT / 3.0)

        def prep_fast(fs, g0, g1, t):
            # g = g0 + t*g1  (DVE)
            V.scalar_tensor_tensor(g[:, :], g1[:, :], float(t), g0[:, :],
                                   OP.mult, OP.add)
            # dl = denom-1 = 2g+EPS interior, g+EPS at row boundaries  (ACT)
            nc.scalar.activation(dl[:, :], g[:, :], AF.Identity,
                                 bias=eps_b[:, 0:1], scale=2.0)
            nc.scalar.activation(dl[:, 0::N], g[:, 0::N], AF.Identity,
                                 bias=eps_b[:, 0:1], scale=1.0)
            nc.scalar.activation(dl[:, N - 1::N], g[:, N - 1::N], AF.Identity,
                                 bias=eps_b[:, 0:1], scale=1.0)
            # r = 1 - dl + dl^2 == (dl - 0.5)^2 + 0.75, entirely on ScalarE
            nc.scalar.activation(tmp[:, :], dl[:, :], AF.Square,
                                 bias=mhalf_b[:, 0:1], scale=1.0)
            nc.scalar.activation(fs["r"][:, :], tmp[:, :], AF.Identity,
                                 bias=b34[:, 0:1], scale=1.0)
            V.tensor_mul(fs["pb"][:, :], g[:, :], fs["r"][:, :])
            nc.scalar.copy(fs["pf"][:, :], fs["pb"][:, :])
            V.memset(fs["pb"][:, N - 1::N], 0.0)
            V.memset(fs["pf"][:, 0::N], 0.0)

        def prep(fs, base, tcoef, t, dt_eff):
            """Build r = 1/denom and the scan coefficient fields phi = g*r."""
            dtf = dt_eff / 3.0
            # ct = max((tcoef*t + base) * dtf, EPS*dtf); the /3 of the moving
            # average and the dt/dx^2 scale are folded in up front.
            V.scalar_tensor_tensor(ct[:, :], tcoef[:, :], float(t), base[:, :],
                                   OP.mult, OP.add)
            V.tensor_scalar(ct[:, :], ct[:, :], float(dtf), float(EPS * dtf),
                            OP.mult, OP.max)
            # g = moving-average smooth along the solve axis (replicate pad).
            V.tensor_tensor(tmp[:, 1:F - 1], ct[:, 0:F - 2], ct[:, 2:F], OP.add)
            V.tensor_tensor(g[:, 1:F - 1], tmp[:, 1:F - 1], ct[:, 1:F - 1], OP.add)
            V.scalar_tensor_tensor(g[:, 0::N], ct[:, 0::N], 2.0, ct[:, 1::N],
                                   OP.mult, OP.add)
            V.scalar_tensor_tensor(g[:, N - 1::N], ct[:, N - 1::N], 2.0,
                                   ct[:, N - 2::N], OP.mult, OP.add)
            # dl = denom - 1 = 2g + EPS - g_i*g_{i-1}, with boundary rows of the
            # tridiagonal having diag 1+g instead of 1+2g.
            V.tensor_scalar(dl[:, :], g[:, :], 2.0, float(EPS), OP.mult, OP.add)
            V.tensor_tensor(tmp[:, 1:F], g[:, 1:F], g[:, 0:F - 1], OP.mult)
            V.tensor_sub(dl[:, 1:F], dl[:, 1:F], tmp[:, 1:F])
            V.tensor_scalar(dl[:, 0::N], g[:, 0::N], float(EPS), None, OP.add)
            V.tensor_sub(dl[:, N - 1::N], dl[:, N - 1::N], g[:, N - 1::N])
            # r = 1 - dl*(1 - dl)
            V.tensor_scalar(tmp[:, :], dl[:, :], -1.0, 1.0, OP.mult, OP.add)
            V.tensor_mul(tmp[:, :], dl[:, :], tmp[:, :])
            V.tensor_scalar(fs["r"][:, :], tmp[:, :], -1.0, 1.0, OP.mult, OP.add)
            # phi = g*r; forward variant zeroes row starts, backward row ends.
            V.tensor_mul(fs["pb"][:, :], g[:, :], fs["r"][:, :])
            nc.scalar.copy(fs["pf"][:, :], fs["pb"][:, :])
            V.memset(fs["pb"][:, N - 1::N], 0.0)
            V.memset(fs["pf"][:, 0::N], 0.0)

        def solve(buf, fs):
            for b in range(BL):
                bb = buf[:, b]
                V.tensor_mul(s_t[:, :], bb, fs["r"][:, :])
                V.tensor_tensor_scan(e_t[:, :], fs["pf"][:, :], s_t[:, :], 0.0,
                                     OP.mult, OP.add)
                V.tensor_tensor_scan(bb[:, ::-1], fs["pb"][:, ::-1],
                                     e_t[:, ::-1], 0.0, OP.mult, OP.add)

        def transpose_vol(src, dst):
            for b in range(BL):
                for c in range(C):
                    pt = psum.tile([128, 512], f32, tag="pt")
                    for sb in range(2):
                        for db in range(2):
                            nc.tensor.transpose(
                                pt[:, db * 256 + sb * 128:db * 256 + (sb + 1) * 128],
                                src[:, b, (c * 2 + sb) * N + db * 128:
                                    (c * 2 + sb) * N + (db + 1) * 128],
                                ident[:, :])
                    nc.scalar.copy(dst[:, b, c * 2 * N:(c * 2 + 2) * N], pt[:, :])

        def couple(src, dst, mbase):
            srcv = src[:, :, :].rearrange("p b (c x) -> p b c x", c=C)
            dstv = dst[:, :, :].rearrange("p b (c x) -> p b c x", c=C)
            for d in range(C):
                nc.scalar.mul(dstv[:, :, d, :], srcv[:, :, 0, :],
                              scal[:, mbase + d * 3:mbase + d * 3 + 1])
                for cc in range(1, C):
                    V.scalar_tensor_tensor(
                        dstv[:, :, d, :], srcv[:, :, cc, :],
                        scal[:, mbase + d * 3 + cc:mbase + d * 3 + cc + 1],
                        dstv[:, :, d, :], OP.mult, OP.add)

        def prep_x(fs, t):
            if fast:
                prep_fast(fs, g0x, g1x, t)
            else:
                prep(fs, ab, atc, t, DT / 2)

        def prep_y(fs, t):
            if fast:
                prep_fast(fs, g0y, g1y, t)
            else:
                prep(fs, bbt, btct, t, DT)

        cur, other = A, Bt
        xcache_t = None
        xping = 0
        t = 0.0
        for step in range(NUM_STEPS):
            if xcache_t != t:
                xping ^= 1
                prep_x(fx[xping], t)
                xcache_t = t
            solve(cur, fx[xping])
            t += DT / 2
            prep_y(fy, t)
            transpose_vol(cur, other)
            solve(other, fy)
            transpose_vol(other, cur)
            t += DT / 2
            if xcache_t != t:
                xping ^= 1
                prep_x(fx[xping], t)
                xcache_t = t
            solve(cur, fx[xping])
            if not diag:
                mbase = 0 if step < NUM_STEPS - 1 else 9
                couple(cur, other, mbase)
                cur, other = other, cur

        # skip blend: out = s*orig + (1-s)*u_final.  Generic path folded (1-s)
        # into the last coupling matrix; diag path applies the commuted
        # per-channel factor (1-s)*m_c^NUM_STEPS here instead.
        for b in range(BL):
            nc.sync.dma_start(out=other[:, b], in_=_chw_ap(u_d, b))
        if diag:
            for b in range(BL):
                for c in range(C):
                    sl = slice(c * 2 * N, (c + 1) * 2 * N)
                    nc.scalar.mul(cur[:, b, sl], cur[:, b, sl],
                                  scal[:, 20 + c:21 + c])
        for b in range(BL):
            V.scalar_tensor_tensor(other[:, b], other[:, b], scal[:, 18:19],
                                   cur[:, b], OP.mult, OP.add)
            nc.sync.dma_start(out=_chw_ap(y_d, b), in_=other[:, b])

    nc.compile()
    return nc


def _fast_ok(alpha_base, beta_base, alpha_time_coeff, beta_time_coeff):
    """Fast path assumes max(base + t*coef, EPS) never clamps for any solve
    time t in [0, NUM_STEPS*DT]."""
    tmax = NUM_STEPS * DT
    for base, tc in ((alpha_base, alpha_time_coeff), (beta_base, beta_time_coeff)):
        base = np.asarray(base, np.float64)
        tc = np.asarray(tc, np.float64)
        lo = np.minimum(base, np.minimum(base + tmax * tc, base + 0.005 * tc))
        if lo.min() <= 10 * EPS:
            return False
    return True


def _run(in_maps, trace=False, fast=True, diag=False, **kw):
    key = (fast, diag)
    if key not in _cached_nc:
        _cached_nc[key] = _build(fast, diag)
    return run_bass_kernel_spmd(_cached_nc[key], in_maps, list(range(NCORES)),
                                trace=trace, **kw)


def _make_in_maps(u, alpha_base, beta_base, alpha_time_coeff, beta_time_coeff,
                  channel_coupling, skip_weight):
    u = np.ascontiguousarray(np.asarray(u, np.float32))
    s = 1.0 / (1.0 + np.exp(-float(np.asarray(skip_weight, np.float64))))
    M = np.asarray(channel_coupling, np.float32)
    row = np.zeros(24, np.float32)
    row[0:9] = M.reshape(-1)
    row[9:18] = (np.float32(1.0 - s) * M).reshape(-1)
    row[18] = np.float32(s)
    row[20:23] = np.float32(1.0 - s) * (np.diag(M).astype(np.float64)
                                        ** NUM_STEPS).astype(np.float32)
    scal = np.ascontiguousarray(np.tile(row[None, :], (128, 1)))
    ab = np.ascontiguousarray(np.asarray(alpha_base, np.float32))
    atc = np.ascontiguousarray(np.asarray(alpha_time_coeff, np.float32))
    bbt = np.ascontiguousarray(np.swapaxes(np.asarray(beta_base, np.float32), 1, 2))
    btct = np.ascontiguousarray(np.swapaxes(np.asarray(beta_time_coeff, np.float32), 1, 2))
    return [dict(u=np.ascontiguousarray(u[i * BL:(i + 1) * BL]), ab=ab, atc=atc,
                 bbt=bbt, btct=btct, scal=scal) for i in range(NCORES)]


def _diag_ok(channel_coupling):
    M = np.asarray(channel_coupling, np.float64)
    return bool(np.all(M == np.diag(np.diag(M))))


# ---------------------------------------------------------------------------
# Matmul path.
#
# The tridiagonal coefficient fields are alpha(t) = alpha_base + t*coeff with
# alpha_base spatially constant in the target regime, so the per-row solve
# matrices T(t) = (1+EPS)I + gamma(t)*A (A = Neumann graph Laplacian along the
# solve axis) are, to ~1e-6 absolute in gamma, the SAME matrix for every row,
# channel and batch.  All solves are then polynomials in the single fixed A:
# x-solves act along w, y-solves along h, they all commute, and (for diagonal
# channel coupling, or channel-uniform gamma with any coupling) the coupling
# commutes through as well.  The whole NUM_STEPS loop collapses to
#     out[b,c] = s*u[b,c] + sum_d CC[c,d] * (Gy_d @ u[b,d] @ Gx_d)
# with Gx = prod_t Tx(t)^-1 (20 factors), Gy = prod_t Ty(t)^-1 (10 factors)
# computed exactly on the host in float64 via one eigendecomposition of A.
# The device does two dense 256x256 bf16 matmuls per image on the PE array.
# Validity of the constant-coefficient approximation is checked on the host;
# on failure we fall back to the exact scan kernel above.
# ---------------------------------------------------------------------------

_a_eig_cache = None


def _a_eig():
    global _a_eig_cache
    if _a_eig_cache is None:
        A = np.zeros((N, N), np.float64)
        idx = np.arange(N)
        A[idx, idx] = 2.0
        A[0, 0] = A[N - 1, N - 1] = 1.0
        A[idx[:-1], idx[:-1] + 1] = -1.0
        A[idx[1:], idx[1:] - 1] = -1.0
        lam, U = np.linalg.eigh(A)
        _a_eig_cache = (lam, U)
    return _a_eig_cache


def _smooth_last(f):
    p = np.pad(f, ((0, 0), (0, 0), (1, 1)), mode="edge")
    return (p[:, :, :-2] + p[:, :, 1:-1] + p[:, :, 2:]) / 3.0


def _mm_params(alpha_base, beta_base, alpha_time_coeff, beta_time_coeff,
               channel_coupling, skip_weight, dev_tol=2e-5):
    """Host-side Gx/Gy construction; returns None when the constant-coefficient
    collapse is not valid for these inputs."""
    ab = np.asarray(alpha_base, np.float64)
    bb = np.asarray(beta_base, np.float64)
    atc = np.asarray(alpha_time_coeff, np.float64)
    btc = np.asarray(beta_time_coeff, np.float64)
    Mcc = np.asarray(channel_coupling, np.float64)
    s = 1.0 / (1.0 + np.exp(-float(np.asarray(skip_weight, np.float64))))

    x_times = [(0.0, 1)] + [(k * DT, 2) for k in range(1, NUM_STEPS)] \
        + [(NUM_STEPS * DT, 1)]
    y_times = [((k + 0.5) * DT, 1) for k in range(NUM_STEPS)]

    def gammas(base, tc, times, dt_eff, swap):
        out = {}
        maxdev = 0.0
        for t, _m in times:
            f = base + tc * t
            if f.min() <= 10 * EPS:      # max(.,EPS) clamp could be active
                return None, None
            if swap:
                f = np.swapaxes(f, 1, 2)
            coeff = _smooth_last(f) * dt_eff / DX ** 2
            g = coeff.mean(axis=(1, 2))
            maxdev = max(maxdev, float(np.abs(coeff - g[:, None, None]).max()))
            out[t] = g
        return out, maxdev

    gxs, dev_x = gammas(ab, atc, x_times, DT / 2, swap=False)
    if gxs is None:
        return None
    gys, dev_y = gammas(bb, btc, y_times, DT, swap=True)
    if gys is None:
        return None
    if max(dev_x, dev_y) > dev_tol:
        return None

    diagcc = bool(np.all(Mcc == np.diag(np.diag(Mcc))))
    if not diagcc:
        dev_c = max(
            max(float(np.abs(g - g.mean()).max()) for g in gxs.values()),
            max(float(np.abs(g - g.mean()).max()) for g in gys.values()))
        if dev_c > dev_tol:
            return None

    lam, U = _a_eig()

    def gmat(times, gdict, c):
        d = np.ones(N)
        for t, m in times:
            d = d / (1.0 + EPS + gdict[t][c] * lam) ** m
        return (U * d) @ U.T

    gx = np.stack([gmat(x_times, gxs, c) for c in range(C)])
    gy = np.stack([gmat(y_times, gys, c) for c in range(C)])

    cc10 = np.linalg.matrix_power(Mcc, NUM_STEPS)
    cc_row = np.zeros(16, np.float64)
    cc_row[0] = s
    if diagcc:
        gx = gx * ((1.0 - s) * np.diag(cc10))[:, None, None]
    else:
        gx = np.broadcast_to(gx.mean(axis=0), (C, N, N))
        gy = np.broadcast_to(gy.mean(axis=0), (C, N, N))
        cc_row[1:10] = ((1.0 - s) * cc10).reshape(-1)
    return dict(gx=np.ascontiguousarray(gx), gy=np.ascontiguousarray(gy),
                diagcc=diagcc, cc=cc_row)


def _build_mm(diagcc=True):
    nc = bacc.Bacc("TRN2", target_bir_lowering=False, debug=False,
                   num_devices=NCORES)
    ut_d = nc.dram_tensor("ut", [BL, C, N, N], bf16, kind="ExternalInput")
    u32_d = nc.dram_tensor("u32", [BL, C, N, N], f32, kind="ExternalInput")
    gx_d = nc.dram_tensor("gx", [C, N, N], bf16, kind="ExternalInput")
    gy_d = nc.dram_tensor("gy", [C, N, N], bf16, kind="ExternalInput")
    cc_d = nc.dram_tensor("cc", [128, 16], f32, kind="ExternalInput")
    y_d = nc.dram_tensor("y", [BL, C, N, N], f32, kind="ExternalOutput")

    with TileContext(nc) as tc, ExitStack() as ctx:
        sb = ctx.enter_context(tc.tile_pool(name="sb", bufs=1))
        psum = ctx.enter_context(tc.tile_pool(name="psum", bufs=4, space="PSUM"))
        psum2 = ctx.enter_context(tc.tile_pool(name="psum2", bufs=2, space="PSUM"))
        V = nc.vector

        gx_sb = sb.tile([128, C, 2, N], bf16, tag="gx")
        gy_sb = sb.tile([128, C, 2, N], bf16, tag="gy")
        cc_sb = sb.tile([128, 16], f32, tag="cc")
        nc.sync.dma_start(out=gx_sb[:, :], in_=_chw_ap(gx_d))
        nc.sync.dma_start(out=gy_sb[:, :], in_=_chw_ap(gy_d))
        nc.sync.dma_start(out=cc_sb[:, :], in_=cc_d.ap())

        ut_sb = sb.tile([128, BL, C, 2, N], bf16, tag="ut")
        u32_sb = sb.tile([128, BL, C, 2, N], f32, tag="u32")
        m1_sb = sb.tile([128, BL, C, 2, N], bf16, tag="m1")
        out_sb = sb.tile([128, BL, C, 2, N], f32, tag="out")
        for b in range(BL):
            nc.sync.dma_start(out=ut_sb[:, b], in_=_chw_ap(ut_d, b))
        for b in range(BL):
            nc.sync.dma_start(out=u32_sb[:, b], in_=_chw_ap(u32_d, b))

        # Stage 1: M1[b,c] = u[b,c] @ Gx_c.  Stationary = transposed-u chunk
        # (so the result lands h-major), moving = Gx chunk rows.
        for b in range(BL):
            for c in range(C):
                pm1 = psum.tile([128, 2, N], f32, tag="pm1")
                for hb in range(2):
                    for kb in range(2):
                        nc.tensor.matmul(
                            pm1[:, hb],
                            ut_sb[:, b, c, kb, hb * 128:(hb + 1) * 128],
                            gx_sb[:, c, kb],
                            start=(kb == 0), stop=(kb == 1))
                nc.scalar.copy(m1_sb[:, b, c], pm1[:, :])

        # Stage 2: V[b,c] = Gy_c @ M1[b,c]; blend out = s*u + (CC mix of V).
        if diagcc:
            for c in range(C):
                for p in range(BL // 2):
                    pp2 = psum2.tile([128, 2, 2, N], f32, tag="pp2")
                    for hb2 in range(2):
                        for kb in range(2):
                            nc.tensor.matmul(
                                pp2[:, hb2],
                                gy_sb[:, c, kb, hb2 * 128:(hb2 + 1) * 128],
                                m1_sb[:, 2 * p:2 * p + 2, c, kb, :],
                                start=(kb == 0), stop=(kb == 1))
                    for i in range(2):
                        b = 2 * p + i
                        V.scalar_tensor_tensor(
                            out_sb[:, b, c], u32_sb[:, b, c], cc_sb[:, 0:1],
                            pp2[:, :, i, :], OP.mult, OP.add)
        else:
            for b in range(BL):
                pv = [psum.tile([128, 2, N], f32, tag=f"pv{c}")
                      for c in range(C)]
                for c in range(C):
                    for hb2 in range(2):
                        for kb in range(2):
                            nc.tensor.matmul(
                                pv[c][:, hb2],
                                gy_sb[:, c, kb, hb2 * 128:(hb2 + 1) * 128],
                                m1_sb[:, b, c, kb, :],
                                start=(kb == 0), stop=(kb == 1))
                for d in range(C):
                    nc.scalar.mul(out_sb[:, b, d], pv[0][:, :],
                                  cc_sb[:, 1 + d * 3:2 + d * 3])
                    for c2 in range(1, C):
                        V.scalar_tensor_tensor(
                            out_sb[:, b, d], pv[c2][:, :],
                            cc_sb[:, 1 + d * 3 + c2:2 + d * 3 + c2],
                            out_sb[:, b, d], OP.mult, OP.add)
                    V.scalar_tensor_tensor(
                        out_sb[:, b, d], u32_sb[:, b, d], cc_sb[:, 0:1],
                        out_sb[:, b, d], OP.mult, OP.add)
        for b in range(BL):
            nc.sync.dma_start(out=_chw_ap(y_d, b), in_=out_sb[:, b])

    nc.compile()
    return nc


def _run_mm(in_maps, diagcc, trace=False, **kw):
    key = ("mm", diagcc)
    if key not in _cached_nc:
        _cached_nc[key] = _build_mm(diagcc)
    return run_bass_kernel_spmd(_cached_nc[key], in_maps, list(range(NCORES)),
                                trace=trace, **kw)


def _make_in_maps_mm(u, params):
    u = np.asarray(u, np.float32)
    ut = np.ascontiguousarray(
        np.swapaxes(u, 2, 3).astype(ml_dtypes.bfloat16))
    u32 = np.ascontiguousarray(u)
    gxb = np.ascontiguousarray(params["gx"].astype(ml_dtypes.bfloat16))
    gyb = np.ascontiguousarray(params["gy"].astype(ml_dtypes.bfloat16))
    cc = np.ascontiguousarray(
        np.tile(params["cc"].astype(np.float32)[None, :], (128, 1)))
    return [dict(ut=ut[i * BL:(i + 1) * BL], u32=u32[i * BL:(i + 1) * BL],
                 gx=gxb, gy=gyb, cc=cc) for i in range(NCORES)]


def kernel(u, alpha_base, beta_base, alpha_time_coeff, beta_time_coeff,
           channel_coupling, skip_weight):
    params = _mm_params(alpha_base, beta_base, alpha_time_coeff,
                        beta_time_coeff, channel_coupling, skip_weight)
    if params is not None:
        in_maps = _make_in_maps_mm(u, params)
        res = _run_mm(in_maps, params["diagcc"])
    else:
        in_maps = _make_in_maps(u, alpha_base, beta_base, alpha_time_coeff,
                                beta_time_coeff, channel_coupling, skip_weight)
        fast = _fast_ok(alpha_base, beta_base, alpha_time_coeff,
                        beta_time_coeff)
        diag = _diag_ok(channel_coupling)
        res = _run(in_maps, fast=fast, diag=diag)
    return np.concatenate([res.results[i]["y"] for i in range(NCORES)], axis=0)

